# revision 8
# baseline (speedup 1.0000x reference)
"""Multi-head QKV block attention for Trainium2, SPMD over 8 NeuronCores.

Problem: X[4,2048,1024], residual[4,2048,1024], wq/wk/wv[1024,1024],
H=16 heads, D=64, softmax scale sqrt(S/H)=sqrt(128).
out = softmax((X wq)(X wk)^T / sqrt(128)) (X wv) + residual, returned twice.

Sharding: core c handles batch b=c//2 and head group g=c%2 (8 heads = 512
feature columns). Fully data/tensor-parallel -- no collectives; host
assembles the output. X is pre-transposed and bf16-cast on the host
(input marshaling) so the device never spends PE time transposing it.

Per-core kernel (Tile framework), fully fused single phase:
  - K/V/Q projections run on the PE with weights/xT as stationary
    operands, interleaved into the attention stream (K-proj for head pair
    m and V-proj for key block b are injected into earlier iterations'
    t-loops so the PE never drains and ScalarE/VectorE are never idle).
  - per head pair (hp) and 512-query tile (st): transposed logits
    K_h^T.T @ Q_h^T as two K=64 matmuls in disjoint PE row groups; exp is
    split across engines per t-chunk: ScalarE spline exp for 7 chunks,
    VectorE Schraudolph bit-trick exp (tensor_scalar mult+add to int16,
    reinterpreted as bf16 -- exp2 via the exponent field) for 9 chunks,
    so neither engine paces the loop; effect^T accumulated as
    [v|1].T @ expT in bf16 (ones row gives the softmax denominator);
    PE-transpose back, normalize on ScalarE, add residual, DMA out.
  - PSUM: one 3-deep ring of [128,1024] fp32 tiles (6 banks) shared by
    logits, projection accumulators and the transpose scratch (bitcast
    view), + 2 banks for the effect accumulators.
  - The t-loop is software-pipelined (logits(t+1) issued before
    effect(t)) so the PE streams ahead of the exp engines.
"""

import math
import sys

for _p in ("/opt/trn_rl_repo", "/root/.axon_site/_ro/trn_rl_repo"):
    if _p not in sys.path:
        sys.path.append(_p)

import numpy as np

B, S, F = 4, 2048, 1024
H = 16
D = 64
G = 512            # feature columns per core (8 heads)
NH = 8             # heads per core
KC = 8             # contraction chunks of 128 over F
ST = 4             # s tiles of 512
TC = 16            # t chunks of 128
SCALE = 1.0 / math.sqrt(S / H)
# Schraudolph exp2-bit-trick constants for bf16 output: round(x*SM + SB) as
# int16 reinterpreted as bf16 approximates exp(SCALE*x). 2^7 scales into the
# bf16 exponent field; c=0.0579 centers the sawtooth error (~1.5% rms).
SCH_C = 0.0579
SCH_M = 128.0 * math.log2(math.e) * SCALE
SCH_B = 128.0 * (127.0 - SCH_C)
DVE_T = frozenset((0, 2, 4, 6, 8, 10, 12, 14, 15))  # exp chunks on VectorE

_cached = None


def _build():
    import concourse.bacc as bacc
    import concourse.tile as tile
    from concourse import mybir
    from concourse.masks import make_identity

    dt = mybir.dt
    AF = mybir.ActivationFunctionType

    nc = bacc.Bacc("TRN2", target_bir_lowering=False, debug=False, num_devices=8)

    xt_d = nc.dram_tensor("xt", [F, S], dt.bfloat16, kind="ExternalInput").ap()
    wq_d = nc.dram_tensor("wq", [F, G], dt.bfloat16, kind="ExternalInput").ap()
    wk_d = nc.dram_tensor("wk", [F, G], dt.bfloat16, kind="ExternalInput").ap()
    wv_d = nc.dram_tensor("wv", [F, G], dt.bfloat16, kind="ExternalInput").ap()
    res_d = nc.dram_tensor("res", [S, G], dt.float32, kind="ExternalInput").ap()
    out_d = nc.dram_tensor("out", [S, G], dt.float32, kind="ExternalOutput").ap()

    with tile.TileContext(nc) as tc:
        with tc.tile_pool(name="persist", bufs=1) as persist:
            identB = persist.tile([128, 128], dt.bfloat16)
            make_identity(nc, identB[:])
            ones = persist.tile([128, NH], dt.float32)
            nc.vector.memset(ones[:], 1.0)
            # Preload the exp table set on ScalarE while the DMAs run -- the
            # ~2.7us ACT_TABLE_LOAD otherwise lands mid-stream and idles PE
            # past the HAM window.
            scr = persist.tile([128, NH], dt.float32)
            nc.scalar.activation(scr[:], ones[:], AF.Exp)

            xT = [persist.tile([128, S], dt.bfloat16, name=f"xT{k}") for k in range(KC)]
            kT = [persist.tile([128, S], dt.bfloat16, name=f"kT{m}") for m in range(4)]
            vS = [persist.tile([128, NH, D + 1], dt.bfloat16, name=f"vS{t}")
                  for t in range(TC)]

            w_sb = {}
            with tc.tile_pool(name="wp", bufs=1) as wp:
                # Weights on the gpsimd DMA queue: wk first (gates K-proj),
                # then wq (Q-proj at loop start), then wv.
                for nm, wd in (("k", wk_d), ("q", wq_d), ("v", wv_d)):
                    for k in range(KC):
                        t = wp.tile([128, G], dt.bfloat16, name=f"w{nm}{k}")
                        nc.gpsimd.dma_start(t[:], wd[k * 128:(k + 1) * 128, :])
                        w_sb[nm, k] = t
                # X^T pieces in consumption order: all k-chunks of key block
                # b before block b+1, so K-proj/V-proj of early key blocks
                # can start while the rest streams in.
                for b in range(4):
                    eng = nc.sync if b % 2 == 0 else nc.gpsimd
                    for k in range(KC):
                        eng.dma_start(
                            xT[k][:, b * 512:(b + 1) * 512],
                            xt_d[k * 128:(k + 1) * 128, b * 512:(b + 1) * 512])

                with tc.tile_pool(name="pp", bufs=3, space="PSUM") as pp, \
                     tc.tile_pool(name="epp", bufs=2, space="PSUM") as epp, \
                     tc.tile_pool(name="qtsp", bufs=8) as qtsp, \
                     tc.tile_pool(name="expa", bufs=4) as expa, \
                     tc.tile_pool(name="expd", bufs=4) as expd, \
                     tc.tile_pool(name="esp", bufs=4) as esp, \
                     tc.tile_pool(name="stp", bufs=8) as stp, \
                     tc.tile_pool(name="rsp", bufs=3) as rsp, \
                     tc.tile_pool(name="rcp", bufs=4) as rcp:

                    def ptile():
                        return pp.tile([128, 1024], dt.float32, tag="lp", name="lp")

                    def emit_kproj(m, b):
                        pk = ptile()
                        for k in range(KC):
                            nc.tensor.matmul(
                                pk[:, 0:512], w_sb["k", k][:, m * 128:(m + 1) * 128],
                                xT[k][:, b * 512:(b + 1) * 512],
                                start=(k == 0), stop=(k == KC - 1))
                        nc.vector.tensor_copy(kT[m][:, b * 512:(b + 1) * 512],
                                              pk[:, 0:512])

                    def emit_vproj_j(b, j):
                        pv = ptile()
                        tci = b * 4 + j
                        for k in range(KC):
                            nc.tensor.matmul(
                                pv[:, 0:512],
                                xT[k][:, tci * 128:(tci + 1) * 128],
                                w_sb["v", k][:], start=(k == 0), stop=(k == KC - 1))
                        nc.vector.tensor_copy(
                            vS[tci][:, :, D:D + 1],
                            ones[:].rearrange("p (h o) -> p h o", o=1))
                        nc.vector.tensor_copy(
                            vS[tci][:, :, 0:D],
                            pv[:, 0:512].rearrange("p (h d) -> p h d", h=NH))

                    def emit_qproj(dst_st, m):
                        pq = ptile()
                        for k in range(KC):
                            nc.tensor.matmul(
                                pq[:, 0:512], w_sb["q", k][:, m * 128:(m + 1) * 128],
                                xT[k][:, dst_st * 512:(dst_st + 1) * 512],
                                start=(k == 0), stop=(k == KC - 1))
                        qt = qtsp.tile([128, 512], dt.bfloat16, tag="qts", name="qt")
                        nc.vector.tensor_copy(qt[:], pq[:, 0:512])
                        return qt

                    def emit_logits(hp, qts, t):
                        # one 2-bank psum tile holds both halves' logits for
                        # this t-chunk: the two K=64 matmuls run concurrently
                        # in disjoint PE row groups.
                        lp = ptile()
                        for half in range(2):
                            r0 = half * 64
                            nc.tensor.matmul(
                                lp[:, half * 512:(half + 1) * 512],
                                kT[hp][r0:r0 + 64, t * 128:(t + 1) * 128],
                                qts[r0:r0 + 64, :],
                                start=True, stop=True)
                        # exp: alternate engines so neither paces the loop.
                        if t in DVE_T:
                            exi = expd.tile([128, 1024], dt.int16, tag="exd", name="exi")
                            nc.vector.tensor_scalar(
                                exi[:], lp[:], SCH_M, SCH_B,
                                mybir.AluOpType.mult, mybir.AluOpType.add)
                            return exi[:].bitcast(dt.bfloat16)
                        ex = expa.tile([128, 1024], dt.bfloat16, tag="exa", name="ex")
                        nc.scalar.activation(ex[:], lp[:], AF.Exp, scale=SCALE)
                        return ex[:]

                    # ---- prologue: K-proj m=0, Q-proj st=0, V-proj b=0 ----
                    for b in range(4):
                        emit_kproj(0, b)
                    qts_cur = [emit_qproj(0, m) for m in range(4)]
                    for j in range(4):
                        emit_vproj_j(0, j)

                    # Remaining projections injected into the st=0 t-loops,
                    # each before its first consumer.
                    inject = {
                        (0, 0): {1: [lambda j=j: emit_vproj_j(1, j) for j in range(2)],
                                 3: [lambda: emit_vproj_j(1, 2), lambda: emit_vproj_j(1, 3),
                                     lambda: emit_kproj(1, 0)],
                                 5: [lambda: emit_vproj_j(2, 0), lambda: emit_vproj_j(2, 1)],
                                 7: [lambda: emit_vproj_j(2, 2), lambda: emit_vproj_j(2, 3),
                                     lambda: emit_kproj(1, 1)],
                                 9: [lambda: emit_vproj_j(3, 0), lambda: emit_vproj_j(3, 1)],
                                 11: [lambda: emit_vproj_j(3, 2), lambda: emit_vproj_j(3, 3),
                                      lambda: emit_kproj(1, 2)],
                                 13: [lambda: emit_kproj(1, 3)]},
                        (0, 1): {1: [lambda: emit_kproj(2, 0)],
                                 5: [lambda: emit_kproj(2, 1)],
                                 9: [lambda: emit_kproj(2, 2)],
                                 13: [lambda: emit_kproj(2, 3)]},
                        (0, 2): {1: [lambda: emit_kproj(3, 0)],
                                 5: [lambda: emit_kproj(3, 1)],
                                 9: [lambda: emit_kproj(3, 2)],
                                 13: [lambda: emit_kproj(3, 3)]},
                    }

                    for st in range(ST):
                        s0 = st * 512
                        # Prefetch this tile's residual rows early.
                        rts = []
                        for j in range(4):
                            rt = rsp.tile([128, G], dt.float32, tag="res", name="rt")
                            nc.sync.dma_start(
                                rt[:], res_d[s0 + j * 128:s0 + (j + 1) * 128, :])
                            rts.append(rt)
                        qts_next = [None] * 4
                        stage = [stp.tile([128, G], dt.float32, tag="stage", name="stage")
                                 for _ in range(4)]
                        for hp in range(4):
                            if st < ST - 1:
                                qts_next[hp] = emit_qproj(st + 1, hp)
                            inj = inject.get((st, hp), {})
                            eps = [epp.tile([D + 1, 512], dt.float32, tag="ep", name="ep")
                                   for _ in range(2)]
                            # software pipeline: logits(t+1) is emitted before
                            # effect(t) so the PE streams logits while the exp
                            # engines work, instead of stalling on ex(t).
                            ex_prev = emit_logits(hp, qts_cur[hp], 0)
                            for t in range(TC):
                                for fn in inj.get(t, ()):
                                    fn()
                                ex_next = (emit_logits(hp, qts_cur[hp], t + 1)
                                           if t < TC - 1 else None)
                                for half in range(2):
                                    nc.tensor.matmul(
                                        eps[half][:],
                                        vS[t][:, 2 * hp + half, :],
                                        ex_prev[:, half * 512:(half + 1) * 512],
                                        start=(t == 0), stop=(t == TC - 1))
                                ex_prev = ex_next
                            # epilogue: cast effect^T to bf16 (ScalarE),
                            # transpose both halves into a psum-ring scratch
                            # (bitcast view), normalize by the ones-row
                            # denominator on ScalarE, write into stage.
                            ess = []
                            for half in range(2):
                                es = esp.tile([D + 1, 512], dt.bfloat16, tag="es", name="es")
                                nc.scalar.copy(es[:], eps[half][:])
                                ess.append(es)
                            tp8 = ptile()[:, 0:264].bitcast(dt.bfloat16) \
                                .rearrange("p (h c) -> p h c", c=D + 2)
                            for half in range(2):
                                for j in range(4):
                                    nc.tensor.transpose(
                                        tp8[:, half * 4 + j, 0:D + 1],
                                        ess[half][:, j * 128:(j + 1) * 128],
                                        identB[0:D + 1, 0:D + 1])
                            rec = rcp.tile([128, 8], dt.float32, tag="rec", name="rec")
                            nc.vector.reciprocal(rec[:], tp8[:, :, D])
                            for half in range(2):
                                h = 2 * hp + half
                                for j in range(4):
                                    idx = half * 4 + j
                                    nc.scalar.activation(
                                        stage[j][:, h * 64:(h + 1) * 64],
                                        tp8[:, idx, 0:D], AF.Copy,
                                        scale=rec[:, idx:idx + 1])
                        qts_cur = qts_next
                        for j in range(4):
                            nc.vector.tensor_add(stage[j][:], stage[j][:], rts[j][:])
                            nc.sync.dma_start(
                                out_d[s0 + j * 128:s0 + (j + 1) * 128, :], stage[j][:])

    nc.compile()
    return nc


def _get_nc():
    global _cached
    if _cached is None:
        _cached = _build()
    return _cached


def _make_in_maps(X, residual_score, wq, wk, wv):
    import ml_dtypes

    bf16 = ml_dtypes.bfloat16
    X = np.asarray(X, dtype=np.float32)
    residual_score = np.ascontiguousarray(np.asarray(residual_score, dtype=np.float32))
    wq = np.asarray(wq, dtype=np.float32).astype(bf16)
    wk = np.asarray(wk, dtype=np.float32).astype(bf16)
    wv = np.asarray(wv, dtype=np.float32).astype(bf16)
    xts = [np.ascontiguousarray(X[b].T.astype(bf16)) for b in range(B)]
    in_maps = []
    for c in range(8):
        b, g = c // 2, c % 2
        cols = slice(g * G, (g + 1) * G)
        in_maps.append({
            "xt": xts[b],
            "wq": np.ascontiguousarray(wq[:, cols]),
            "wk": np.ascontiguousarray(wk[:, cols]),
            "wv": np.ascontiguousarray(wv[:, cols]),
            "res": np.ascontiguousarray(residual_score[b, :, cols]),
        })
    return in_maps


def _assemble(results):
    out = np.empty((B, S, F), dtype=np.float32)
    for c in range(8):
        b, g = c // 2, c % 2
        out[b, :, g * G:(g + 1) * G] = results[c]["out"]
    return out


def run(X, residual_score, wq, wk, wv, trace=False):
    from concourse.bass_utils import run_bass_kernel_spmd

    nc = _get_nc()
    in_maps = _make_in_maps(X, residual_score, wq, wk, wv)
    res = run_bass_kernel_spmd(nc, in_maps, core_ids=list(range(8)), trace=trace)
    return _assemble(res.results), res


def kernel(X, residual_score, wq, wk, wv):
    out, _ = run(X, residual_score, wq, wk, wv)
    return (out, out)


# revision 12
# speedup vs baseline: 1.0488x; 1.0488x over previous
"""Multi-head QKV block attention for Trainium2, SPMD over 8 NeuronCores.

Problem: X[4,2048,1024], residual[4,2048,1024], wq/wk/wv[1024,1024],
H=16 heads, D=64, softmax scale sqrt(S/H)=sqrt(128).
out = softmax((X wq)(X wk)^T / sqrt(128)) (X wv) + residual, returned twice.

Sharding: core c handles batch b=c//2 and head group g=c%2 (8 heads = 512
feature columns). Fully data/tensor-parallel -- no collectives; host
assembles the output. X is pre-transposed and bf16-cast on the host
(input marshaling) so the device never spends PE time transposing it.

Per-core kernel (Tile framework), fully fused single phase:
  - K/V/Q projections run on the PE with weights/xT as stationary
    operands, interleaved into the attention stream (K-proj for head pair
    m and V-proj for key block b are injected into earlier iterations'
    t-loops so the PE never drains and ScalarE/VectorE are never idle).
  - per head pair (hp) and 512-query tile (st): transposed logits
    K_h^T.T @ Q_h^T as two K=64 matmuls in disjoint PE row groups; exp is
    split across engines per t-chunk: ScalarE spline exp for 7 chunks,
    VectorE Schraudolph bit-trick exp (tensor_scalar mult+add to int16,
    reinterpreted as bf16 -- exp2 via the exponent field) for 9 chunks,
    so neither engine paces the loop; effect^T accumulated as
    [v|1].T @ expT in bf16 (ones row gives the softmax denominator);
    PE-transpose back, normalize on ScalarE, add residual, DMA out.
  - PSUM: one 3-deep ring of [128,1024] fp32 tiles (6 banks) shared by
    logits, projection accumulators and the transpose scratch (bitcast
    view), + 2 banks for the effect accumulators.
  - The t-loop is software-pipelined (logits(t+1) issued before
    effect(t)) so the PE streams ahead of the exp engines.
"""

import math
import sys

for _p in ("/opt/trn_rl_repo", "/root/.axon_site/_ro/trn_rl_repo"):
    if _p not in sys.path:
        sys.path.append(_p)

import numpy as np

B, S, F = 4, 2048, 1024
H = 16
D = 64
G = 512            # feature columns per core (8 heads)
NH = 8             # heads per core
KC = 8             # contraction chunks of 128 over F
ST = 4             # s tiles of 512
TC = 16            # t chunks of 128
SCALE = 1.0 / math.sqrt(S / H)
# Schraudolph exp2-bit-trick constants for bf16 output: round(x*SM + SB) as
# int16 reinterpreted as bf16 approximates exp(SCALE*x). 2^7 scales into the
# bf16 exponent field; c=0.0579 centers the sawtooth error (~1.5% rms).
SCH_C = 0.0579
SCH_M = 128.0 * math.log2(math.e) * SCALE
SCH_B = 128.0 * (127.0 - SCH_C)
# VectorE takes fewer exp chunks than ScalarE because it also owns the
# per-head-pair epilogue (cast/reciprocal/normalize) and the proj copies.
DVE_T = frozenset((2, 5, 8, 11, 14, 15))

_cached = None


def _build():
    import concourse.bacc as bacc
    import concourse.tile as tile
    from concourse import mybir
    from concourse.masks import make_identity

    dt = mybir.dt
    AF = mybir.ActivationFunctionType

    nc = bacc.Bacc("TRN2", target_bir_lowering=False, debug=False, num_devices=8)

    xt_d = nc.dram_tensor("xt", [F, S], dt.bfloat16, kind="ExternalInput").ap()
    wq_d = nc.dram_tensor("wq", [F, G], dt.bfloat16, kind="ExternalInput").ap()
    wk_d = nc.dram_tensor("wk", [F, G], dt.bfloat16, kind="ExternalInput").ap()
    wv_d = nc.dram_tensor("wv", [F, G], dt.bfloat16, kind="ExternalInput").ap()
    res_d = nc.dram_tensor("res", [S, G], dt.float32, kind="ExternalInput").ap()
    out_d = nc.dram_tensor("out", [S, G], dt.float32, kind="ExternalOutput").ap()

    with tile.TileContext(nc) as tc:
        with tc.tile_pool(name="persist", bufs=1) as persist:
            identB = persist.tile([128, 128], dt.bfloat16)
            make_identity(nc, identB[:])
            ones = persist.tile([128, NH], dt.float32)
            nc.vector.memset(ones[:], 1.0)
            # Preload the exp table set on ScalarE while the DMAs run -- the
            # ~2.7us ACT_TABLE_LOAD otherwise lands mid-stream and idles PE
            # past the HAM window.
            scr = persist.tile([128, NH], dt.float32)
            nc.scalar.activation(scr[:], ones[:], AF.Exp)

            xT = [persist.tile([128, S], dt.bfloat16, name=f"xT{k}") for k in range(KC)]
            kT = [persist.tile([128, S], dt.bfloat16, name=f"kT{m}") for m in range(4)]
            vS = [persist.tile([128, NH, D + 1], dt.bfloat16, name=f"vS{t}")
                  for t in range(TC)]

            w_sb = {}
            with tc.tile_pool(name="wp", bufs=1) as wp:
                # Weights on the gpsimd DMA queue: wk first (gates K-proj),
                # then wq (Q-proj at loop start), then wv.
                for nm, wd in (("k", wk_d), ("q", wq_d), ("v", wv_d)):
                    for k in range(KC):
                        t = wp.tile([128, G], dt.bfloat16, name=f"w{nm}{k}")
                        nc.gpsimd.dma_start(t[:], wd[k * 128:(k + 1) * 128, :])
                        w_sb[nm, k] = t
                # X^T pieces in consumption order: all k-chunks of key block
                # b before block b+1, so K-proj/V-proj of early key blocks
                # can start while the rest streams in.
                for b in range(4):
                    eng = nc.sync if b % 2 == 0 else nc.gpsimd
                    for k in range(KC):
                        eng.dma_start(
                            xT[k][:, b * 512:(b + 1) * 512],
                            xt_d[k * 128:(k + 1) * 128, b * 512:(b + 1) * 512])

                # PSUM: lp ring 3x2 banks (logits / proj accumulators / tp8
                # transpose scratch) + eps 2x1 banks = 8.
                with tc.tile_pool(name="pp", bufs=3, space="PSUM") as pp, \
                     tc.tile_pool(name="epp", bufs=2, space="PSUM") as epp, \
                     tc.tile_pool(name="qtsp", bufs=8) as qtsp, \
                     tc.tile_pool(name="expa", bufs=4) as expa, \
                     tc.tile_pool(name="expd", bufs=4) as expd, \
                     tc.tile_pool(name="esp", bufs=4) as esp, \
                     tc.tile_pool(name="stp", bufs=8) as stp, \
                     tc.tile_pool(name="rsp", bufs=3) as rsp, \
                     tc.tile_pool(name="rcp", bufs=4) as rcp:

                    def ptile():
                        return pp.tile([128, 1024], dt.float32, tag="lp", name="lp")

                    def emit_kproj(m, b):
                        pk = ptile()
                        for k in range(KC):
                            nc.tensor.matmul(
                                pk[:, 0:512], w_sb["k", k][:, m * 128:(m + 1) * 128],
                                xT[k][:, b * 512:(b + 1) * 512],
                                start=(k == 0), stop=(k == KC - 1))
                        nc.vector.tensor_copy(kT[m][:, b * 512:(b + 1) * 512],
                                              pk[:, 0:512])

                    def emit_vproj_j(b, j):
                        pv = ptile()
                        tci = b * 4 + j
                        for k in range(KC):
                            nc.tensor.matmul(
                                pv[:, 0:512],
                                xT[k][:, tci * 128:(tci + 1) * 128],
                                w_sb["v", k][:], start=(k == 0), stop=(k == KC - 1))
                        nc.vector.tensor_copy(
                            vS[tci][:, :, D:D + 1],
                            ones[:].rearrange("p (h o) -> p h o", o=1))
                        nc.vector.tensor_copy(
                            vS[tci][:, :, 0:D],
                            pv[:, 0:512].rearrange("p (h d) -> p h d", h=NH))

                    def emit_qproj(dst_st, m):
                        pq = ptile()
                        for k in range(KC):
                            nc.tensor.matmul(
                                pq[:, 0:512], w_sb["q", k][:, m * 128:(m + 1) * 128],
                                xT[k][:, dst_st * 512:(dst_st + 1) * 512],
                                start=(k == 0), stop=(k == KC - 1))
                        qt = qtsp.tile([128, 512], dt.bfloat16, tag="qts", name="qt")
                        nc.vector.tensor_copy(qt[:], pq[:, 0:512])
                        return qt

                    def emit_logits(hp, qts, t):
                        # one 2-bank psum tile holds both halves' logits for
                        # this t-chunk: the two K=64 matmuls run concurrently
                        # in disjoint PE row groups.
                        lp = ptile()
                        for half in range(2):
                            r0 = half * 64
                            nc.tensor.matmul(
                                lp[:, half * 512:(half + 1) * 512],
                                kT[hp][r0:r0 + 64, t * 128:(t + 1) * 128],
                                qts[r0:r0 + 64, :],
                                start=True, stop=True)
                        # exp: alternate engines so neither paces the loop.
                        if t in DVE_T:
                            exi = expd.tile([128, 1024], dt.int16, tag="exd", name="exi")
                            nc.vector.tensor_scalar(
                                exi[:], lp[:], SCH_M, SCH_B,
                                mybir.AluOpType.mult, mybir.AluOpType.add)
                            return exi[:].bitcast(dt.bfloat16)
                        ex = expa.tile([128, 1024], dt.bfloat16, tag="exa", name="ex")
                        nc.scalar.activation(ex[:], lp[:], AF.Exp, scale=SCALE)
                        return ex[:]

                    # ---- prologue: K-proj m=0, Q-proj st=0, V-proj b=0 ----
                    for b in range(4):
                        emit_kproj(0, b)
                    qts_cur = [emit_qproj(0, m) for m in range(4)]
                    for j in range(4):
                        emit_vproj_j(0, j)

                    # Remaining projections injected into the st=0 t-loops,
                    # one per t slot, each emitted before its first consumer:
                    # vproj(b,j) before effect(4b+j) of the NEXT head pair,
                    # kproj(m,*) anywhere before C(0,m) starts.
                    inject = {
                        (0, 0): dict(
                            [(4 * (b - 1) + j, [lambda b=b, j=j: emit_vproj_j(b, j)])
                             for b in range(1, 4) for j in range(4)] +
                            [(12 + b, [lambda b=b: emit_kproj(1, b)]) for b in range(4)]),
                        (0, 1): {2 + 4 * b: [lambda b=b: emit_kproj(2, b)]
                                 for b in range(4)},
                        (0, 2): {2 + 4 * b: [lambda b=b: emit_kproj(3, b)]
                                 for b in range(4)},
                    }

                    for st in range(ST):
                        s0 = st * 512
                        # Prefetch this tile's residual rows early.
                        rts = []
                        for j in range(4):
                            rt = rsp.tile([128, G], dt.float32, tag="res", name="rt")
                            nc.sync.dma_start(
                                rt[:], res_d[s0 + j * 128:s0 + (j + 1) * 128, :])
                            rts.append(rt)
                        qts_next = [None] * 4
                        stage = [stp.tile([128, G], dt.float32, tag="stage", name="stage")
                                 for _ in range(4)]
                        for hp in range(4):
                            if st < ST - 1:
                                qts_next[hp] = emit_qproj(st + 1, hp)
                            inj = inject.get((st, hp), {})
                            eps = [epp.tile([D + 1, 512], dt.float32, tag="ep", name="ep")
                                   for _ in range(2)]
                            # software pipeline: logits(t+1) is emitted before
                            # effect(t) so the PE streams logits while the exp
                            # engines work, instead of stalling on ex(t).
                            ex_prev = emit_logits(hp, qts_cur[hp], 0)
                            for t in range(TC):
                                for fn in inj.get(t, ()):
                                    fn()
                                ex_next = (emit_logits(hp, qts_cur[hp], t + 1)
                                           if t < TC - 1 else None)
                                for half in range(2):
                                    nc.tensor.matmul(
                                        eps[half][:],
                                        vS[t][:, 2 * hp + half, :],
                                        ex_prev[:, half * 512:(half + 1) * 512],
                                        start=(t == 0), stop=(t == TC - 1))
                                ex_prev = ex_next
                            # epilogue (all VectorE, so cross-engine waits
                            # never sit at the head of the exp engines'
                            # FIFOs): cast effect^T to bf16, PE-transpose
                            # both halves into a psum-ring scratch (bitcast
                            # view), normalize by the ones-row denominator,
                            # write into stage.
                            ess = []
                            for half in range(2):
                                es = esp.tile([D + 1, 512], dt.bfloat16, tag="es", name="es")
                                nc.vector.tensor_copy(es[:], eps[half][:])
                                ess.append(es)
                            tp8 = ptile()[:, 0:264].bitcast(dt.bfloat16) \
                                .rearrange("p (h c) -> p h c", c=D + 2)
                            for half in range(2):
                                for j in range(4):
                                    nc.tensor.transpose(
                                        tp8[:, half * 4 + j, 0:D + 1],
                                        ess[half][:, j * 128:(j + 1) * 128],
                                        identB[0:D + 1, 0:D + 1])
                            rec = rcp.tile([128, 8], dt.float32, tag="rec", name="rec")
                            nc.vector.reciprocal(rec[:], tp8[:, :, D])
                            for half in range(2):
                                h = 2 * hp + half
                                for j in range(4):
                                    idx = half * 4 + j
                                    nc.vector.tensor_scalar_mul(
                                        stage[j][:, h * 64:(h + 1) * 64],
                                        tp8[:, idx, 0:D],
                                        rec[:, idx:idx + 1])
                        qts_cur = qts_next
                        for j in range(4):
                            nc.vector.tensor_add(stage[j][:], stage[j][:], rts[j][:])
                            nc.sync.dma_start(
                                out_d[s0 + j * 128:s0 + (j + 1) * 128, :], stage[j][:])

    nc.compile()
    return nc


def _get_nc():
    global _cached
    if _cached is None:
        _cached = _build()
    return _cached


def _make_in_maps(X, residual_score, wq, wk, wv):
    import ml_dtypes

    bf16 = ml_dtypes.bfloat16
    X = np.asarray(X, dtype=np.float32)
    residual_score = np.ascontiguousarray(np.asarray(residual_score, dtype=np.float32))
    wq = np.asarray(wq, dtype=np.float32).astype(bf16)
    wk = np.asarray(wk, dtype=np.float32).astype(bf16)
    wv = np.asarray(wv, dtype=np.float32).astype(bf16)
    xts = [np.ascontiguousarray(X[b].T.astype(bf16)) for b in range(B)]
    in_maps = []
    for c in range(8):
        b, g = c // 2, c % 2
        cols = slice(g * G, (g + 1) * G)
        in_maps.append({
            "xt": xts[b],
            "wq": np.ascontiguousarray(wq[:, cols]),
            "wk": np.ascontiguousarray(wk[:, cols]),
            "wv": np.ascontiguousarray(wv[:, cols]),
            "res": np.ascontiguousarray(residual_score[b, :, cols]),
        })
    return in_maps


def _assemble(results):
    out = np.empty((B, S, F), dtype=np.float32)
    for c in range(8):
        b, g = c // 2, c % 2
        out[b, :, g * G:(g + 1) * G] = results[c]["out"]
    return out


def run(X, residual_score, wq, wk, wv, trace=False):
    from concourse.bass_utils import run_bass_kernel_spmd

    nc = _get_nc()
    in_maps = _make_in_maps(X, residual_score, wq, wk, wv)
    res = run_bass_kernel_spmd(nc, in_maps, core_ids=list(range(8)), trace=trace)
    return _assemble(res.results), res


def kernel(X, residual_score, wq, wk, wv):
    out, _ = run(X, residual_score, wq, wk, wv)
    return (out, out)
